# revision 1
# baseline (speedup 1.0000x reference)
"""Multi-head self-attention Trainium2 kernel.

Sharding: 8 cores = 2 batches x 4 head-groups. Core c handles batch c//4 and
heads [4g, 4g+4) where g = c%4 (dims [256g, 256g+256) of the 1024 model dim).

Per-core device program (matmul operands float32r -> full speed at N>=256
with ~13-bit effective mantissa; accumulation fp32 in PSUM):
  - QT/KT projections computed transposed: QT[d, t] = Wq_g @ x_b^T (+bias; Q
    additionally scaled by 1/sqrt(64)); V computed in natural [token, dim]
    layout and stored with a ones-column appended per head (65 cols).
  - Attention per (dtile, q-chunk): S^T tiles for the two heads of the dtile
    land in one 2-bank PSUM tile -> a single wide exp on the scalar engine,
    then per head ctxu^T[65, q] += Vaug_h^T @ expS^T, so row 64 accumulates
    the softmax denominator for free.
  - Normalization: DVE reciprocal of row 64, gpsimd partition_broadcast to 64
    partitions, DVE multiply into ctx^T.
  - Out-projection: out_partial = ctx_g @ W_out[:, dims_g]^T (row-parallel).

Host: shards/transposes inputs, sums the 4 partial outputs per batch and adds
b_out + b_v @ W_out^T (the V-bias contribution commutes through softmax since
attention rows sum to 1).
"""

import numpy as np

import concourse.bacc as bacc
import concourse.mybir as mybir
from concourse.tile import TileContext
from concourse.bass_utils import run_bass_kernel_spmd

AF = mybir.ActivationFunctionType
ALU = mybir.AluOpType
F32 = mybir.dt.float32
F32R = mybir.dt.float32r
# compute dtype for matmul operands: float32r runs at bf16 speed for N>=256
# on trn2 with ~13-bit effective mantissa (measured on HW)
CDT = F32R
NP_CDT = np.float32

B, S, D, H, DH = 2, 2048, 1024, 16, 64
DG = 256          # dims per head-group (4 heads)
TC = 512          # token / query chunk
NTC = S // TC     # 4
NTT = S // 128    # 16 token tiles
NKT = S // 128    # 16 key tiles

_NC_CACHE = None


def _build_nc():
    nc = bacc.Bacc("TRN2", target_bir_lowering=False, debug=False)

    xT = nc.dram_tensor("xT", [D, S], CDT, kind="ExternalInput")
    wq = nc.dram_tensor("wqT", [D, DG], CDT, kind="ExternalInput")
    wk = nc.dram_tensor("wkT", [D, DG], CDT, kind="ExternalInput")
    wv = nc.dram_tensor("wvT", [D, DG], CDT, kind="ExternalInput")
    wo = nc.dram_tensor("woT", [DG, D], CDT, kind="ExternalInput")
    bq = nc.dram_tensor("bq", [2, 128], F32, kind="ExternalInput")
    bk = nc.dram_tensor("bk", [2, 128], F32, kind="ExternalInput")
    out = nc.dram_tensor("out", [S, D], F32, kind="ExternalOutput")

    with TileContext(nc) as tc:
        with (
            tc.tile_pool(name="const", bufs=1) as constp,
            tc.tile_pool(name="xt", bufs=3) as xtp,
            tc.tile_pool(name="expst", bufs=4) as expp,
            tc.tile_pool(name="small", bufs=4) as smallp,
            tc.tile_pool(name="outp", bufs=3) as outp,
            tc.tile_pool(name="s_ps", bufs=3, space="PSUM") as sps,
            tc.tile_pool(name="ctx_ps", bufs=2, space="PSUM") as ctxps,
        ):
            # ---- persistent tiles ----
            wq_s = constp.tile([128, 8, DG], CDT)
            wk_s = constp.tile([128, 8, DG], CDT)
            wv_s = constp.tile([128, 8, DG], CDT)
            bq_s = constp.tile([128, 2], F32)
            bk_s = constp.tile([128, 2], F32)
            nc.sync.dma_start(out=bq_s, in_=bq[:, :].rearrange("t p -> p t"))
            nc.sync.dma_start(out=bk_s, in_=bk[:, :].rearrange("t p -> p t"))
            # per-k-chunk weight + first-x-chunk loads, interleaved so the
            # first projection matmuls start as soon as their slices land
            wqr = wq[:, :].rearrange("(k p) m -> p k m", p=128)
            wkr = wk[:, :].rearrange("(k p) m -> p k m", p=128)
            wvr = wv[:, :].rearrange("(k p) m -> p k m", p=128)
            xTr0 = xT[:, :].rearrange("(k p) t -> p k t", p=128)
            xt0 = xtp.tile([128, 8, TC], CDT, name="xt0", tag="xt")
            for k in range(8):
                nc.sync.dma_start(out=wq_s[:, k, :], in_=wqr[:, k, :])
                nc.sync.dma_start(out=wk_s[:, k, :], in_=wkr[:, k, :])
                nc.sync.dma_start(out=xt0[:, k, :], in_=xTr0[:, k, 0:TC])
                nc.sync.dma_start(out=wv_s[:, k, :], in_=wvr[:, k, :])

            QT_s = constp.tile([128, 2, S], CDT)
            # K stored zero-padded per head: head hh of dtile d lives in
            # partitions [64*hh, 64*hh+64) of KTz_s[:, d, hh, :], zeros
            # elsewhere. Score matmuls then contract over K=128 against the
            # full packed Q slice (zeros annihilate the other head's rows),
            # so every matmul in the kernel runs in plain 128-row mode --
            # no row-tiling, no PE mode-switch drains (measured ~56ns/key-tile)
            KTz_s = constp.tile([128, 2, 2, S], CDT)
            nc.gpsimd.memset(KTz_s[:, :, :, :].bitcast(F32), 0.0)
            ctxT_s = constp.tile([128, 2, S], CDT)
            Vg_s = constp.tile([128, NTT, 4, 65], CDT)
            nc.vector.memset(Vg_s[:, :, :, :].bitcast(F32), 1.0)

            # warm the PE clock (HAM / p-state ramp) with dummy matmuls while
            # the initial DMAs stream in, so real matmuls start at full rate
            warm = constp.tile([128, TC], CDT)
            nc.vector.memset(warm[:, :].bitcast(F32), 1.0)
            wps = sps.tile([128, TC], F32, tag="s", name="wps")
            for _ in range(10):
                nc.tensor.matmul(wps, lhsT=warm[:, 0:128], rhs=warm,
                                 start=True, stop=True)

            xTr = xT[:, :].rearrange("(k p) t -> p k t", p=128)

            # ---- projection group emitters (q/k transposed, v natural) ----
            def emit_q_group(d, tci, xt):
                tsl = slice(tci * TC, (tci + 1) * TC)
                dsl = slice(d * 128, (d + 1) * 128)
                psq = sps.tile([128, TC], F32, tag="s", name="psq")
                for k in range(8):
                    nc.tensor.matmul(psq, lhsT=wq_s[:, k, dsl], rhs=xt[:, k, :],
                                     start=(k == 0), stop=(k == 7))
                nc.vector.tensor_scalar(QT_s[:, d, tsl], psq,
                                        scalar1=bq_s[:, d:d + 1], scalar2=0.125,
                                        op0=ALU.add, op1=ALU.mult)

            def emit_k_group(d, tci, xt):
                tsl = slice(tci * TC, (tci + 1) * TC)
                dsl = slice(d * 128, (d + 1) * 128)
                psk = sps.tile([128, TC], F32, tag="s", name="psk")
                for k in range(8):
                    nc.tensor.matmul(psk, lhsT=wk_s[:, k, dsl], rhs=xt[:, k, :],
                                     start=(k == 0), stop=(k == 7))
                for hh in range(2):
                    p0 = 64 * hh
                    nc.vector.tensor_scalar(KTz_s[p0:p0 + 64, d, hh, tsl],
                                            psk[p0:p0 + 64, :],
                                            scalar1=bk_s[p0:p0 + 64, d:d + 1],
                                            scalar2=None, op0=ALU.add)

            def emit_v_group(tci, tt, xt):
                ti = tci * 4 + tt
                psv = sps.tile([128, DG], F32, tag="s", name="psv")
                for k in range(8):
                    nc.tensor.matmul(psv, lhsT=xt[:, k, tt * 128:(tt + 1) * 128],
                                     rhs=wv_s[:, k, :], start=(k == 0), stop=(k == 7))
                for h in range(4):
                    nc.vector.tensor_copy(Vg_s[:, ti, h, 0:64],
                                          psv[:, h * 64:(h + 1) * 64])

            # ---- phase B: all projections, one 512-token chunk at a time ----
            for tci in range(NTC):
                if tci == 0:
                    xt = xt0
                else:
                    xt = xtp.tile([128, 8, TC], CDT, tag="xt")
                    nc.sync.dma_start(out=xt, in_=xTr[:, :, tci * TC:(tci + 1) * TC])
                for dd in range(2):
                    emit_q_group(dd, tci, xt)
                    emit_k_group(dd, tci, xt)
                for tt in range(4):
                    emit_v_group(tci, tt, xt)

            # ---- phase C: attention; out-projection groups drip into the PE
            # gaps of the ACT-bound kt loops as their q-chunks complete ----
            wo_s = constp.tile([128, 2, D], CDT)
            nc.sync.dma_start(out=wo_s, in_=wo[:, :].rearrange("(k p) m -> p k m", p=128))

            def emit_d_group(tt, oc, evict_engine=None):
                psl = slice(tt * 128, (tt + 1) * 128)
                osl = slice(oc * TC, (oc + 1) * TC)
                po = sps.tile([128, TC], F32, tag="s", name="po")
                for d in range(2):
                    nc.tensor.matmul(po, lhsT=ctxT_s[:, d, psl],
                                     rhs=wo_s[:, d, osl],
                                     start=(d == 0), stop=(d == 1))
                ot = outp.tile([128, TC], F32)
                if evict_engine is None:
                    nc.vector.tensor_copy(ot, po)
                else:
                    evict_engine.copy(ot, po)
                nc.sync.dma_start(out=out[psl, osl], in_=ot)

            # flat stream over (unit, key-tile): ST+exp always one step ahead
            # of PV, continuing straight across unit boundaries so the scalar
            # engine never drains between q-chunks
            units = [(qc, d) for qc in range(NTC) for d in range(2)]
            cps_of = {}
            drip = []

            def emit_st(u, kt):
                qc, d = units[u]
                qsl = slice(qc * TC, (qc + 1) * TC)
                ksl = slice(kt * 128, (kt + 1) * 128)
                sp = sps.tile([128, 2, TC], F32, tag="s")
                for hh in range(2):
                    nc.tensor.matmul(sp[:, hh, :], lhsT=KTz_s[:, d, hh, ksl],
                                     rhs=QT_s[:, d, qsl],
                                     start=True, stop=True)
                ex = expp.tile([128, 2, TC], CDT)
                nc.scalar.activation(ex, sp, AF.Exp)
                return ex

            def emit_pv(u, kt, ex):
                qc, d = units[u]
                qsl = slice(qc * TC, (qc + 1) * TC)
                if kt == 0:
                    cps_of[u] = [ctxps.tile([128, TC], F32, tag="ctx",
                                            name=f"ctx{hh}") for hh in range(2)]
                cps = cps_of[u]
                for hh in range(2):
                    h = 2 * d + hh
                    nc.tensor.matmul(cps[hh][0:65, :], lhsT=Vg_s[:, kt, h, :],
                                     rhs=ex[:, hh, :],
                                     start=(kt == 0), stop=(kt == NKT - 1))
                if kt == NKT - 1:
                    for hh in range(2):
                        p0 = 64 * hh
                        rec = smallp.tile([1, TC], F32, tag="rec")
                        nc.vector.reciprocal(rec, cps[hh][64:65, :])
                        rbs = smallp.tile([64, TC], F32, tag="rbs")
                        nc.gpsimd.partition_broadcast(rbs, rec[0:1, :], channels=64)
                        nc.vector.tensor_mul(ctxT_s[p0:p0 + 64, d, qsl],
                                             cps[hh][0:64, :], rbs)
                    del cps_of[u]
                    if d == 1:
                        drip.extend((tt, oc)
                                    for tt in range(qc * 4, (qc + 1) * 4)
                                    for oc in range(2))

            prev = None
            for u in range(len(units)):
                for kt in range(NKT):
                    ex = emit_st(u, kt)
                    if prev is not None:
                        emit_pv(*prev)
                    if drip and kt % 2 == 1:
                        emit_d_group(*drip.pop(0))
                    prev = (u, kt, ex)
            emit_pv(*prev)
            # tail: nothing left to overlap — alternate evictions between the
            # idle scalar engine and DVE to shorten the epilogue
            for i, g in enumerate(drip):
                emit_d_group(*g, evict_engine=nc.scalar if i % 2 == 0 else None)

    nc.finalize()
    return nc


def get_nc():
    global _NC_CACHE
    if _NC_CACHE is None:
        _NC_CACHE = _build_nc()
    return _NC_CACHE


def make_in_maps(x, W_q, b_q, W_k, b_k, W_v, b_v, W_out, b_out):
    xb = [np.ascontiguousarray(x[b].T).astype(NP_CDT) for b in range(B)]
    in_maps = []
    for c in range(8):
        b, g = divmod(c, 4)
        sl = slice(DG * g, DG * (g + 1))
        in_maps.append({
            "xT": xb[b],
            "wqT": np.ascontiguousarray(W_q[sl, :].T).astype(NP_CDT),
            "wkT": np.ascontiguousarray(W_k[sl, :].T).astype(NP_CDT),
            "wvT": np.ascontiguousarray(W_v[sl, :].T).astype(NP_CDT),
            "woT": np.ascontiguousarray(W_out[:, sl].T).astype(NP_CDT),
            "bq": b_q[sl].reshape(2, 128).astype(np.float32),
            "bk": b_k[sl].reshape(2, 128).astype(np.float32),
        })
    return in_maps


def combine_outputs(outs, W_out, b_out, b_v):
    host_bias = (b_out + b_v @ W_out.T).astype(np.float32)
    y = np.empty((B, S, D), np.float32)
    for b in range(B):
        y[b] = outs[4 * b] + outs[4 * b + 1] + outs[4 * b + 2] + outs[4 * b + 3]
        y[b] += host_bias
    return y


def kernel(x, W_q, b_q, W_k, b_k, W_v, b_v, W_out, b_out):
    x = np.asarray(x, dtype=np.float32)
    args = [np.asarray(a, dtype=np.float32)
            for a in (W_q, b_q, W_k, b_k, W_v, b_v, W_out, b_out)]
    W_q, b_q, W_k, b_k, W_v, b_v, W_out, b_out = args
    nc = get_nc()
    in_maps = make_in_maps(x, W_q, b_q, W_k, b_k, W_v, b_v, W_out, b_out)
    last_err = None
    for attempt in range(3):
        try:
            res = run_bass_kernel_spmd(nc, in_maps, core_ids=list(range(8)))
            break
        except Exception as e:  # transient device-unrecoverable flakes
            last_err = e
            import time
            time.sleep(10)
    else:
        raise last_err
    outs = [r["out"] for r in res.results]
    return combine_outputs(outs, W_out, b_out, b_v)



# revision 7
# speedup vs baseline: 1.1691x; 1.1691x over previous
"""Multi-head self-attention Trainium2 kernel (v2).

Sharding: 8 cores = 2 batches x 4 head-groups. Core c handles batch c//4 and
heads [4g, 4g+4) where g = c%4 (dims [256g, 256g+256) of the 1024 model dim).

v2 design (vs the v1 baseline):
  - all matmul operands bf16 (inputs cast on host; Q/K built via fp32-PSUM
    projections then rounded once). Halves input DMA and SBUF footprint.
  - score matmuls contract over each head's 64 partitions directly (no
    zero-padded K copy needed).
  - PV matmuls flipped: exp(S^T) is the *stationary* operand and the ones-
    augmented bf16 V tile the *moving* operand, so each accumulation step
    streams 65 columns instead of 512 (PV tensor time 54.6us -> 27.7us).
    ctx lands in natural [token, dim] layout in a [128, 8, 128] PSUM tile
    (65-wide groups at 512B offsets; column 64 accumulates the softmax
    denominator). A PE transpose against a bf16 identity restores ctx^T
    for the out-projection.
  - exp (the scalar engine's ~133us of work - the critical engine) starts
    at ~7us: score/exp steps for the first four q-chunk units are
    interleaved into the projection phase as their K/Q chunks land, and the
    tail paces one ST per ~1.5 PV steps so the scalar engine never drains.

Host: shards/transposes inputs, sums the 4 partial outputs per batch and adds
b_out + b_v @ W_out^T (the V-bias contribution commutes through softmax since
attention rows sum to 1).
"""

import numpy as np

import concourse.bacc as bacc
import concourse.mybir as mybir
from concourse.masks import make_identity
from concourse.tile import TileContext
from concourse.bass_utils import run_bass_kernel_spmd

AF = mybir.ActivationFunctionType
ALU = mybir.AluOpType
F32 = mybir.dt.float32
BF16 = mybir.dt.bfloat16
NP_BF16 = mybir.dt.np(BF16)

B, S, D, H, DH = 2, 2048, 1024, 16, 64
DG = 256          # dims per head-group (4 heads)
TC = 512          # token / query chunk
NTC = S // TC     # 4
NKT = S // 128    # 16 key tiles

_NC_CACHE = None


def _build_nc():
    nc = bacc.Bacc("TRN2", target_bir_lowering=False, debug=False)

    xT = nc.dram_tensor("xT", [D, S], BF16, kind="ExternalInput")
    wq = nc.dram_tensor("wqT", [D, DG], BF16, kind="ExternalInput")
    wk = nc.dram_tensor("wkT", [D, DG], BF16, kind="ExternalInput")
    wv = nc.dram_tensor("wvT", [D, DG], BF16, kind="ExternalInput")
    wo = nc.dram_tensor("woT", [DG, D], BF16, kind="ExternalInput")
    bq = nc.dram_tensor("bq", [2, 128], F32, kind="ExternalInput")
    bk = nc.dram_tensor("bk", [2, 128], F32, kind="ExternalInput")
    out = nc.dram_tensor("out", [S, D], F32, kind="ExternalOutput")

    with TileContext(nc) as tc:
        with (
            tc.tile_pool(name="const", bufs=1) as constp,
            tc.tile_pool(name="xt", bufs=2) as xtp,
            tc.tile_pool(name="expst", bufs=46) as expp,
            tc.tile_pool(name="stg", bufs=6) as stgp,
            tc.tile_pool(name="small", bufs=4) as smallp,
            tc.tile_pool(name="outp", bufs=3) as outp,
            tc.tile_pool(name="s_ps", bufs=2, space="PSUM") as sps,
            tc.tile_pool(name="p_ps", bufs=2, space="PSUM") as pps,
            tc.tile_pool(name="c_ps", bufs=1, space="PSUM") as ctxps,
        ):
            # ---- persistent tiles ----
            wq_s = constp.tile([128, 8, DG], BF16)
            wk_s = constp.tile([128, 8, DG], BF16)
            wv_s = constp.tile([128, 8, DG], BF16)
            bq_s = constp.tile([128, 2], F32)
            bk_s = constp.tile([128, 2], F32)
            nc.sync.dma_start(out=bq_s, in_=bq[:, :].rearrange("t p -> p t"))
            nc.sync.dma_start(out=bk_s, in_=bk[:, :].rearrange("t p -> p t"))
            nc.sync.dma_start(out=wq_s, in_=wq[:, :].rearrange("(k p) m -> p k m", p=128))
            nc.sync.dma_start(out=wk_s, in_=wk[:, :].rearrange("(k p) m -> p k m", p=128))
            xTr = xT[:, :].rearrange("(k p) t -> p k t", p=128)
            xt0 = xtp.tile([128, 8, TC], BF16, name="xt0", tag="xt")
            nc.sync.dma_start(out=xt0, in_=xTr[:, :, 0:TC])
            nc.sync.dma_start(out=wv_s, in_=wv[:, :].rearrange("(k p) m -> p k m", p=128))
            wo_s = constp.tile([128, 2, D], BF16)
            nc.sync.dma_start(out=wo_s, in_=wo[:, :].rearrange("(k p) m -> p k m", p=128))

            # QT/KT: heads of dtile d packed along partitions (head hh in
            # partitions [64hh, 64hh+64)); bf16
            QT_s = constp.tile([128, 2, S], BF16)
            KT_s = constp.tile([128, 2, S], BF16)
            ctxT_s = constp.tile([128, 2, S], BF16)
            # V natural layout, bf16, ones-column at index 64 per head
            Vg_s = constp.tile([128, NKT, 4, 65], BF16)
            nc.vector.memset(Vg_s[:, :, :, 64:65], 1.0)
            ident = constp.tile([128, 128], BF16)
            make_identity(nc, ident)

            # warm the PE clock while the initial DMAs stream in
            warm = constp.tile([128, TC], BF16)
            nc.vector.memset(warm, 1.0)
            wps = pps.tile([128, TC], F32, tag="pt", name="wps")
            for _ in range(8):
                nc.tensor.matmul(wps, lhsT=warm[:, 0:128], rhs=warm,
                                 start=True, stop=True)

            # ---- projection group emitters (q/k transposed, v natural) ----
            def emit_q_group(d, tci, xt):
                tsl = slice(tci * TC, (tci + 1) * TC)
                dsl = slice(d * 128, (d + 1) * 128)
                psq = pps.tile([128, TC], F32, tag="pt", name="psq")
                for k in range(8):
                    nc.tensor.matmul(psq, lhsT=wq_s[:, k, dsl], rhs=xt[:, k, :],
                                     start=(k == 0), stop=(k == 7))
                nc.vector.tensor_scalar(QT_s[:, d, tsl], psq,
                                        scalar1=bq_s[:, d:d + 1], scalar2=0.125,
                                        op0=ALU.add, op1=ALU.mult)

            def emit_k_group(d, tci, xt):
                tsl = slice(tci * TC, (tci + 1) * TC)
                dsl = slice(d * 128, (d + 1) * 128)
                psk = pps.tile([128, TC], F32, tag="pt", name="psk")
                for k in range(8):
                    nc.tensor.matmul(psk, lhsT=wk_s[:, k, dsl], rhs=xt[:, k, :],
                                     start=(k == 0), stop=(k == 7))
                nc.vector.tensor_scalar(KT_s[:, d, tsl], psk,
                                        scalar1=bk_s[:, d:d + 1],
                                        scalar2=None, op0=ALU.add)

            def emit_v_group(tci, tt, xt):
                ti = tci * 4 + tt
                psv = pps.tile([128, TC], F32, tag="pt", name="psv")
                for k in range(8):
                    nc.tensor.matmul(psv[:, 0:DG],
                                     lhsT=xt[:, k, tt * 128:(tt + 1) * 128],
                                     rhs=wv_s[:, k, :], start=(k == 0), stop=(k == 7))
                # single strided copy: psv natural [t, 4*64] -> Vg[:, ti, h, 0:64]
                nc.vector.tensor_copy(Vg_s[:, ti, 0:4, 0:64],
                                      psv[:, 0:DG].rearrange("p (h e) -> p h e", h=4))

            # ---- attention emitters ----
            units = [(qc, d) for qc in range(NTC) for d in range(2)]
            ex_of = {}     # (u, kt) -> exp tile (bf16 SBUF)
            cps_of = {}    # u -> PSUM accum tile [128, 8, 128]
            drip = []      # pending out-projection groups

            def emit_st(u, kt):
                qc, d = units[u]
                qsl = slice(qc * TC, (qc + 1) * TC)
                ksl = slice(kt * 128, (kt + 1) * 128)
                sp = sps.tile([128, 2, TC], F32, tag="s", name="sp")
                for hh in range(2):
                    p0 = 64 * hh
                    nc.tensor.matmul(sp[:, hh, :],
                                     lhsT=KT_s[p0:p0 + 64, d, ksl],
                                     rhs=QT_s[p0:p0 + 64, d, qsl],
                                     start=True, stop=True)
                ex = expp.tile([128, 2, TC], BF16, tag="ex", name="ex")
                nc.scalar.activation(ex, sp, AF.Exp)
                ex_of[(u, kt)] = ex

            def emit_pv(u, kt):
                qc, d = units[u]
                ex = ex_of.pop((u, kt))
                if kt == 0:
                    cps_of[u] = ctxps.tile([128, 8, 128], F32, tag="c", name="cps")
                cps = cps_of[u]
                # cps is two PSUM banks (groups 0-3 / 4-7). start=True marks a
                # whole 2KB bank pending-zero, so exactly one start per bank
                # (first group) and one stop per bank (last group).
                for qs in range(4):
                    for hh in range(2):
                        h = 2 * d + hh
                        g = 2 * qs + hh
                        nc.tensor.matmul(cps[:, g, 0:65],
                                         lhsT=ex[:, hh, qs * 128:(qs + 1) * 128],
                                         rhs=Vg_s[:, kt, h, :],
                                         start=(kt == 0 and g % 4 == 0),
                                         stop=(kt == NKT - 1 and g % 4 == 3))
                if kt == NKT - 1:
                    emit_norm(u)

            def emit_norm(u):
                qc, d = units[u]
                cps = cps_of.pop(u)
                rec = smallp.tile([128, 8, 1], F32, tag="rec", name="rec")
                nc.vector.reciprocal(rec, cps[:, :, 64:65])
                for qs in range(4):
                    stage = stgp.tile([128, 2, 64], BF16, tag="stg", name="stage")
                    for hh in range(2):
                        nc.vector.tensor_scalar(stage[:, hh, :],
                                                cps[:, 2 * qs + hh, 0:64],
                                                scalar1=rec[:, 2 * qs + hh, :],
                                                scalar2=None, op0=ALU.mult)
                    tps = pps.tile([128, 128], BF16, tag="pt", name="tps")
                    nc.tensor.transpose(tps,
                                        stage[:, :, :].rearrange("p a b -> p (a b)"),
                                        ident)
                    qsl = slice(qc * TC + qs * 128, qc * TC + (qs + 1) * 128)
                    nc.vector.tensor_copy(ctxT_s[:, d, qsl], tps)
                if d == 1:
                    drip.extend((tt, oc)
                                for tt in range(qc * 4, (qc + 1) * 4)
                                for oc in range(2))

            def emit_d_group(tt, oc):
                psl = slice(tt * 128, (tt + 1) * 128)
                osl = slice(oc * TC, (oc + 1) * TC)
                po = pps.tile([128, TC], F32, tag="pt", name="po")
                for d in range(2):
                    nc.tensor.matmul(po, lhsT=ctxT_s[:, d, psl],
                                     rhs=wo_s[:, d, osl],
                                     start=(d == 0), stop=(d == 1))
                ot = outp.tile([128, TC], F32)
                nc.vector.tensor_copy(ot, po)
                nc.sync.dma_start(out=out[psl, osl], in_=ot)

            # ---- fused projection + early-attention phase ----
            # Per x-chunk tci: project Q/K/V with ST/exp steps interleaved
            # between groups. STs are limited to what the scalar engine can
            # absorb and to Q/K chunks that exist. Only u0's PV trails here
            # (single PSUM accumulator live).
            early_st = {
                0: [(0, k) for k in range(4)] + [(1, k) for k in range(4)],
                1: [(0, k) for k in range(4, 8)] + [(1, k) for k in range(4, 8)]
                   + [(2, k) for k in range(4)] + [(3, k) for k in range(4)],
                2: [(0, k) for k in range(8, 12)] + [(1, k) for k in range(8, 12)]
                   + [(2, k) for k in range(4, 8)] + [(3, k) for k in range(4, 8)],
                3: [(0, k) for k in range(12, 16)] + [(1, k) for k in range(12, 16)]
                   + [(2, k) for k in range(8, 12)] + [(3, k) for k in range(8, 12)],
            }
            early_pv = {}
            for tci in range(NTC):
                if tci == 0:
                    xt = xt0
                else:
                    xt = xtp.tile([128, 8, TC], BF16, tag="xt", name="xt")
                    nc.sync.dma_start(out=xt, in_=xTr[:, :, tci * TC:(tci + 1) * TC])
                # old-kt STs (kt from previous chunks) may run right away;
                # new-kt STs need this chunk's K groups emitted first.
                sts = list(early_st[tci])
                old_sts = [s for s in sts if s[1] < 4 * tci]
                new_sts = [s for s in sts if s[1] >= 4 * tci]
                pvs = list(early_pv.get(tci, []))

                def weave(n_st):
                    for _ in range(n_st):
                        if old_sts:
                            emit_st(*old_sts.pop(0))
                        elif ready_new and new_sts:
                            emit_st(*new_sts.pop(0))
                    if pvs and (old_sts or new_sts or ready_new):
                        emit_pv(*pvs.pop(0))

                ready_new = False
                emit_q_group(0, tci, xt)
                weave(2)
                emit_q_group(1, tci, xt)
                weave(2)
                emit_k_group(0, tci, xt)
                weave(2)
                emit_k_group(1, tci, xt)
                ready_new = True
                weave(2)
                for tt in range(4):
                    emit_v_group(tci, tt, xt)
                    weave(2)
                while old_sts or new_sts:
                    weave(2)
                while pvs:
                    emit_pv(*pvs.pop(0))

            # ---- tail ----
            # remaining STs: last blocks of u2/u3, then u4..u7 in full;
            # remaining PVs: u0's last block, then u1..u7 in full. Pace ST
            # emission at 72 STs / 116 PV steps so the scalar engine stays
            # fed without over-buffering exp tiles.
            st_queue = ([(2, k) for k in range(12, 16)]
                        + [(3, k) for k in range(12, 16)]
                        + [(u, k) for u in range(4, 8) for k in range(NKT)])
            pv_queue = [(u, k) for u in range(0, 8) for k in range(NKT)]
            n_st, n_pv = len(st_queue), len(pv_queue)
            st_done = 0
            for pv_done, pv in enumerate(pv_queue):
                while st_queue and st_done * n_pv <= n_st * pv_done:
                    emit_st(*st_queue.pop(0))
                    st_done += 1
                emit_pv(*pv)
                if pv_done % 2 == 1 and drip:
                    emit_d_group(*drip.pop(0))
            while drip:
                emit_d_group(*drip.pop(0))

    nc.finalize()
    return nc


def get_nc():
    global _NC_CACHE
    if _NC_CACHE is None:
        _NC_CACHE = _build_nc()
    return _NC_CACHE


def make_in_maps(x, W_q, b_q, W_k, b_k, W_v, b_v, W_out, b_out):
    xb = [np.ascontiguousarray(x[b].T).astype(NP_BF16) for b in range(B)]
    in_maps = []
    for c in range(8):
        b, g = divmod(c, 4)
        sl = slice(DG * g, DG * (g + 1))
        in_maps.append({
            "xT": xb[b],
            "wqT": np.ascontiguousarray(W_q[sl, :].T).astype(NP_BF16),
            "wkT": np.ascontiguousarray(W_k[sl, :].T).astype(NP_BF16),
            "wvT": np.ascontiguousarray(W_v[sl, :].T).astype(NP_BF16),
            "woT": np.ascontiguousarray(W_out[:, sl].T).astype(NP_BF16),
            "bq": b_q[sl].reshape(2, 128).astype(np.float32),
            "bk": b_k[sl].reshape(2, 128).astype(np.float32),
        })
    return in_maps


def combine_outputs(outs, W_out, b_out, b_v):
    host_bias = (b_out + b_v @ W_out.T).astype(np.float32)
    y = np.empty((B, S, D), np.float32)
    for b in range(B):
        y[b] = outs[4 * b] + outs[4 * b + 1] + outs[4 * b + 2] + outs[4 * b + 3]
        y[b] += host_bias
    return y


def kernel(x, W_q, b_q, W_k, b_k, W_v, b_v, W_out, b_out):
    x = np.asarray(x, dtype=np.float32)
    args = [np.asarray(a, dtype=np.float32)
            for a in (W_q, b_q, W_k, b_k, W_v, b_v, W_out, b_out)]
    W_q, b_q, W_k, b_k, W_v, b_v, W_out, b_out = args
    nc = get_nc()
    in_maps = make_in_maps(x, W_q, b_q, W_k, b_k, W_v, b_v, W_out, b_out)
    last_err = None
    for attempt in range(3):
        try:
            res = run_bass_kernel_spmd(nc, in_maps, core_ids=list(range(8)))
            break
        except Exception as e:  # transient device-unrecoverable flakes
            last_err = e
            import time
            time.sleep(10)
    else:
        raise last_err
    outs = [np.asarray(r["out"], dtype=np.float32) for r in res.results]
    return combine_outputs(outs, W_out, b_out, b_v)


# revision 43
# speedup vs baseline: 1.3335x; 1.1406x over previous
"""Multi-head self-attention Trainium2 kernel (v2, 166us vs 222us baseline).

Sharding: 8 cores = 2 batches x 4 head-groups. Core c handles batch c//4 and
heads [4g, 4g+4) where g = c%4 (dims [256g, 256g+256) of the 1024 model dim).

Design (timings from the TimelineSim cost model, which charges matmuls by
moving-operand columns only):
  - All matmul operands bf16: inputs cast on host (halves input DMA), Q/K/V
    built via fp32-PSUM projections and rounded once. Output DMA'd as bf16.
  - Score matmuls contract over each head's 64 partitions directly
    (K/Q packed two heads per 128 partitions; no zero-padded K copy).
  - PV flipped: exp(S^T) is the stationary operand, the ones-augmented bf16
    V tile the moving one, so each accumulation step streams 65 columns
    instead of 512 (PV tensor time 54.6us -> 27.7us). ctx accumulates in
    natural [token, dim] layout in one [128, 8, 128] PSUM tile per unit
    (eight 65-wide groups at 512B offsets, exactly one start/stop per 2KB
    PSUM bank; column 64 collects the softmax denominator). A PE transpose
    against a bf16 identity restores ctx^T for the out-projection.
  - The scalar engine's exp stream (~133us busy) is the critical chain:
    score/exp steps for the first four q-chunk units interleave into the
    projection phase as their K/Q chunks land (64 of 128 exp tiles retired
    before projections finish), the tail paces one ST per ~1.5 PV steps,
    and a 60-deep exp-tile pool prevents slot-reuse stalls.
  - PE warmup matmuls bridge the initial DMA wait (the cost model's p-state
    ramp punishes any idle gap before full clock is reached).
  - Normalization reciprocals/multiplies, ctx transposes+copies, and
    out-projection drips are spread across PV steps (transposes age-gated
    two steps behind their stage multiplies so the PE never waits on DVE);
    the endgame rotates evictions over DVE and the scalar engine (gpsimd
    cannot touch PSUM).

Host: shards/transposes inputs, sums the 4 partial outputs per batch and adds
b_out + b_v @ W_out^T (the V-bias contribution commutes through softmax since
attention rows sum to 1).
"""

import numpy as np

import concourse.bacc as bacc
import concourse.mybir as mybir
from concourse.masks import make_identity
from concourse.tile import TileContext
from concourse.bass_utils import run_bass_kernel_spmd

AF = mybir.ActivationFunctionType
I16 = mybir.dt.int16
# Schraudolph exp in bf16-bit space: bf16bits(exp(s)) ~= int16(A*s + B)
SCH_A = 128.0 / float(np.log(2.0))
SCH_B = 127.0 * 128.0 - 7.0 + 0.5
SCH_SET = None  # set below after os import
ALU = mybir.AluOpType
F32 = mybir.dt.float32
BF16 = mybir.dt.bfloat16
NP_BF16 = mybir.dt.np(BF16)

B, S, D, H, DH = 2, 2048, 1024, 16, 64
DG = 256          # dims per head-group (4 heads)
TC = 512          # token / query chunk
NTC = S // TC     # 4
NKT = S // 128    # 16 key tiles

_NC_CACHE = None


def _build_nc():
    nc = bacc.Bacc("TRN2", target_bir_lowering=False, debug=False)

    xT = nc.dram_tensor("xT", [D, S], BF16, kind="ExternalInput")
    wq = nc.dram_tensor("wqT", [D, DG], BF16, kind="ExternalInput")
    wk = nc.dram_tensor("wkT", [D, DG], BF16, kind="ExternalInput")
    wv = nc.dram_tensor("wvT", [D, DG], BF16, kind="ExternalInput")
    wo = nc.dram_tensor("woT", [DG, D], BF16, kind="ExternalInput")
    bq = nc.dram_tensor("bq", [2, 128], F32, kind="ExternalInput")
    bk = nc.dram_tensor("bk", [2, 128], F32, kind="ExternalInput")
    out = nc.dram_tensor("out", [S, D], BF16, kind="ExternalOutput")

    with TileContext(nc) as tc:
        with (
            tc.tile_pool(name="const", bufs=1) as constp,
            tc.tile_pool(name="xt", bufs=2) as xtp,
            tc.tile_pool(name="expst", bufs=60) as expp,
            tc.tile_pool(name="stg", bufs=12) as stgp,
            tc.tile_pool(name="small", bufs=4) as smallp,
            tc.tile_pool(name="outp", bufs=3) as outp,
            tc.tile_pool(name="s_ps", bufs=2, space="PSUM") as sps,
            tc.tile_pool(name="p_ps", bufs=2, space="PSUM") as pps,
            tc.tile_pool(name="c_ps", bufs=1, space="PSUM") as ctxps,
        ):
            # ---- persistent tiles ----
            wq_s = constp.tile([128, 8, DG], BF16)
            wk_s = constp.tile([128, 8, DG], BF16)
            wv_s = constp.tile([128, 8, DG], BF16)
            bq_s = constp.tile([128, 2], F32)
            bk_s = constp.tile([128, 2], F32)
            xTr = xT[:, :].rearrange("(k p) t -> p k t", p=128)
            xt0 = xtp.tile([128, 8, TC], BF16, name="xt0", tag="xt")
            wqr = wq[:, :].rearrange("(k p) m -> p k m", p=128)
            nc.sync.dma_start(out=wq_s[:, 0:4, :], in_=wqr[:, 0:4, :])
            nc.sync.dma_start(out=xt0[:, 0:4, :], in_=xTr[:, 0:4, 0:TC])
            nc.sync.dma_start(out=wq_s[:, 4:8, :], in_=wqr[:, 4:8, :])
            nc.sync.dma_start(out=wk_s, in_=wk[:, :].rearrange("(k p) m -> p k m", p=128))
            nc.sync.dma_start(out=xt0[:, 4:8, :], in_=xTr[:, 4:8, 0:TC])
            nc.sync.dma_start(out=bq_s, in_=bq[:, :].rearrange("t p -> p t"))
            nc.sync.dma_start(out=bk_s, in_=bk[:, :].rearrange("t p -> p t"))
            nc.sync.dma_start(out=wv_s, in_=wv[:, :].rearrange("(k p) m -> p k m", p=128))
            wo_s = constp.tile([128, 2, D], BF16)
            nc.sync.dma_start(out=wo_s, in_=wo[:, :].rearrange("(k p) m -> p k m", p=128))

            # QT/KT: heads of dtile d packed along partitions (head hh in
            # partitions [64hh, 64hh+64)); bf16
            QT_s = constp.tile([128, 2, S], BF16)
            KT_s = constp.tile([128, 2, S], BF16)
            ctxT_s = constp.tile([128, 2, S], BF16)
            # V natural layout, bf16, ones-column at index 64 per head
            Vg_s = constp.tile([128, NKT, 4, 65], BF16)
            nc.vector.memset(Vg_s[:, :, :, 64:65], 1.0)
            ident = constp.tile([128, 128], BF16)
            make_identity(nc, ident)

            # warm the PE clock while the initial DMAs stream in
            warm = constp.tile([128, TC], BF16)
            nc.vector.memset(warm, 1.0)
            wps = pps.tile([128, TC], F32, tag="pt", name="wps")
            for _ in range(8):
                nc.tensor.matmul(wps, lhsT=warm[:, 0:128], rhs=warm,
                                 start=True, stop=True)

            # ---- projection group emitters (q/k transposed, v natural) ----
            def emit_q_group(d, tci, xt):
                tsl = slice(tci * TC, (tci + 1) * TC)
                dsl = slice(d * 128, (d + 1) * 128)
                psq = pps.tile([128, TC], F32, tag="pt", name="psq")
                for k in range(8):
                    nc.tensor.matmul(psq, lhsT=wq_s[:, k, dsl], rhs=xt[:, k, :],
                                     start=(k == 0), stop=(k == 7))
                nc.vector.tensor_scalar(QT_s[:, d, tsl], psq,
                                        scalar1=bq_s[:, d:d + 1], scalar2=0.125,
                                        op0=ALU.add, op1=ALU.mult)

            def emit_k_group(d, tci, xt):
                tsl = slice(tci * TC, (tci + 1) * TC)
                dsl = slice(d * 128, (d + 1) * 128)
                psk = pps.tile([128, TC], F32, tag="pt", name="psk")
                for k in range(8):
                    nc.tensor.matmul(psk, lhsT=wk_s[:, k, dsl], rhs=xt[:, k, :],
                                     start=(k == 0), stop=(k == 7))
                nc.vector.tensor_scalar(KT_s[:, d, tsl], psk,
                                        scalar1=bk_s[:, d:d + 1],
                                        scalar2=None, op0=ALU.add)

            def emit_v_group(tci, tt, xt):
                ti = tci * 4 + tt
                psv = pps.tile([128, TC], F32, tag="pt", name="psv")
                for k in range(8):
                    nc.tensor.matmul(psv[:, 0:DG],
                                     lhsT=xt[:, k, tt * 128:(tt + 1) * 128],
                                     rhs=wv_s[:, k, :], start=(k == 0), stop=(k == 7))
                # single strided copy: psv natural [t, 4*64] -> Vg[:, ti, h, 0:64]
                nc.vector.tensor_copy(Vg_s[:, ti, 0:4, 0:64],
                                      psv[:, 0:DG].rearrange("p (h e) -> p h e", h=4))

            # ---- attention emitters ----
            units = [(qc, d) for qc in range(NTC) for d in range(2)]
            ex_of = {}     # (u, kt) -> exp tile (bf16 SBUF)
            cps_of = {}    # u -> PSUM accum tile [128, 8, 128]
            drip = []      # pending out-projection groups

            def emit_st(u, kt):
                qc, d = units[u]
                qsl = slice(qc * TC, (qc + 1) * TC)
                ksl = slice(kt * 128, (kt + 1) * 128)
                sp = sps.tile([128, 2, TC], F32, tag="s", name="sp")
                for hh in range(2):
                    p0 = 64 * hh
                    nc.tensor.matmul(sp[:, hh, :],
                                     lhsT=KT_s[p0:p0 + 64, d, ksl],
                                     rhs=QT_s[p0:p0 + 64, d, qsl],
                                     start=True, stop=True)
                if (u, kt) in SCH_SET:
                    exi = expp.tile([128, 2, TC], I16, tag="ex", name="exi")
                    nc.vector.tensor_scalar(exi, sp, scalar1=SCH_A,
                                            scalar2=SCH_B,
                                            op0=ALU.mult, op1=ALU.add)
                    ex_of[(u, kt)] = exi.bitcast(BF16)
                else:
                    ex = expp.tile([128, 2, TC], BF16, tag="ex", name="ex")
                    nc.scalar.activation(ex, sp, AF.Exp)
                    ex_of[(u, kt)] = ex

            def emit_pv(u, kt):
                qc, d = units[u]
                ex = ex_of.pop((u, kt))
                if kt == 0:
                    cps_of[u] = ctxps.tile([128, 8, 128], F32, tag="c", name="cps")
                cps = cps_of[u]
                # cps is two PSUM banks (groups 0-3 / 4-7). start=True marks a
                # whole 2KB bank pending-zero, so exactly one start per bank
                # (first group) and one stop per bank (last group).
                for qs in range(4):
                    for hh in range(2):
                        h = 2 * d + hh
                        g = 2 * qs + hh
                        nc.tensor.matmul(cps[:, g, 0:65],
                                         lhsT=ex[:, hh, qs * 128:(qs + 1) * 128],
                                         rhs=Vg_s[:, kt, h, :],
                                         start=(kt == 0 and g % 4 == 0),
                                         stop=(kt == NKT - 1 and g % 4 == 3))
                if kt == NKT - 1:
                    emit_norm(u)

            post = []      # deferred (unit, transpose+copy closure) steps

            def emit_norm(u):
                qc, d = units[u]
                cps = cps_of.pop(u)
                rec = smallp.tile([128, 8, 1], F32, tag="rec", name="rec")
                nc.vector.reciprocal(rec, cps[:, :, 64:65])
                stages = []
                for qs in range(4):
                    stage = stgp.tile([128, 2, 64], BF16, tag="stg", name="stage")
                    for hh in range(2):
                        nc.vector.tensor_scalar(stage[:, hh, :],
                                                cps[:, 2 * qs + hh, 0:64],
                                                scalar1=rec[:, 2 * qs + hh, :],
                                                scalar2=None, op0=ALU.mult)
                    stages.append(stage)

                def mk(qs, stage):
                    def go():
                        tps = pps.tile([128, 128], BF16, tag="pt", name="tps")
                        nc.tensor.transpose(
                            tps, stage[:, :, :].rearrange("p a b -> p (a b)"),
                            ident)
                        qsl = slice(qc * TC + qs * 128, qc * TC + (qs + 1) * 128)
                        if endgame[0] and qs % 2 == 0:
                            nc.scalar.copy(ctxT_s[:, d, qsl], tps)
                        else:
                            nc.vector.tensor_copy(ctxT_s[:, d, qsl], tps)
                    return go
                for qs in range(4):
                    post.append((step_ctr[0], u, qs, mk(qs, stages[qs])))

            done_posts = {}
            released = set()

            def _maybe_release_drips(u, qs=None):
                qc, d = units[u]
                other = u - 1 if d == 1 else u + 1
                for q in ([qs] if qs is not None else range(4)):
                    if (q in done_posts.get(u, set())
                            and q in done_posts.get(other, set())
                            and (u, q) not in released):
                        released.add((u, q))
                        released.add((other, q))
                        drip.extend(((qc * 4 + q, oc, None) for oc in range(2)))

            step_ctr = [0]

            def pop_post(force=False):
                if not post:
                    return
                if not force and step_ctr[0] - post[0][0] < 2:
                    return
                _, u, qs, go = post.pop(0)
                go()
                done_posts.setdefault(u, set()).add(qs)
                _maybe_release_drips(u, qs)

            endgame = [False]
            dripn = [0]

            def emit_d_group(tt, oc, evict="dve", d_only=None):
                psl = slice(tt * 128, (tt + 1) * 128)
                osl = slice(oc * TC, (oc + 1) * TC)
                dripn[0] += 1
                if endgame[0]:
                    evict = ("dve", "act")[dripn[0] % 2]
                po = pps.tile([128, TC], F32, tag="pt", name="po")
                for i, d in enumerate((0, 1)):
                    nc.tensor.matmul(po, lhsT=ctxT_s[:, d, psl],
                                     rhs=wo_s[:, d, osl],
                                     start=(i == 0), stop=(i == 1))
                ot = outp.tile([128, TC], BF16)
                if evict == "act":
                    nc.scalar.copy(ot, po)
                else:
                    nc.vector.tensor_copy(ot, po)
                nc.sync.dma_start(out=out[psl, osl], in_=ot)

            # ---- fused projection + early-attention phase ----
            # Per x-chunk tci: project Q/K/V with ST/exp steps interleaved
            # between groups. STs are limited to what the scalar engine can
            # absorb and to Q/K chunks that exist. Only u0's PV trails here
            # (single PSUM accumulator live).
            early_st = {
                0: [(0, k) for k in range(4)] + [(1, k) for k in range(4)],
                1: [(0, k) for k in range(4, 8)] + [(1, k) for k in range(4, 8)]
                   + [(2, k) for k in range(4)] + [(3, k) for k in range(4)],
                2: [(0, k) for k in range(8, 12)] + [(1, k) for k in range(8, 12)]
                   + [(2, k) for k in range(4, 8)] + [(3, k) for k in range(4, 8)],
                3: [(0, k) for k in range(12, 16)] + [(1, k) for k in range(12, 16)]
                   + [(2, k) for k in range(8, 16)] + [(3, k) for k in range(8, 16)],
            }
            early_pv = {}
            for tci in range(NTC):
                if tci == 0:
                    xt = xt0
                else:
                    xt = xtp.tile([128, 8, TC], BF16, tag="xt", name="xt")
                    nc.sync.dma_start(out=xt, in_=xTr[:, :, tci * TC:(tci + 1) * TC])
                # old-kt STs (kt from previous chunks) may run right away;
                # new-kt STs need this chunk's K groups emitted first.
                sts = list(early_st[tci])
                old_sts = [s for s in sts if s[1] < 4 * tci]
                # new-kt STs split by which K d-group they need
                new0 = [s for s in sts if s[1] >= 4 * tci and units[s[0]][1] == 0]
                new1 = [s for s in sts if s[1] >= 4 * tci and units[s[0]][1] == 1]
                pvs = list(early_pv.get(tci, []))
                ready = [False, False]

                def weave(n_st):
                    for _ in range(n_st):
                        if old_sts:
                            emit_st(*old_sts.pop(0))
                        elif ready[0] and new0:
                            emit_st(*new0.pop(0))
                        elif ready[1] and new1:
                            emit_st(*new1.pop(0))
                    if pvs:
                        emit_pv(*pvs.pop(0))
                        step_ctr[0] += 1
                    elif post:
                        pop_post()

                nflush = 0
                for s in list(old_sts):
                    if nflush >= 3:
                        break
                    if units[s[0]][0] < tci:
                        old_sts.remove(s)
                        emit_st(*s)
                        nflush += 1
                emit_q_group(0, tci, xt)
                emit_k_group(0, tci, xt)
                ready[0] = True
                weave(2)
                emit_q_group(1, tci, xt)
                weave(2)
                emit_k_group(1, tci, xt)
                ready[1] = True
                weave(2)
                for tt in range(4):
                    emit_v_group(tci, tt, xt)
                    weave(2)
                while old_sts or new0 or new1:
                    weave(2)
                while pvs:
                    emit_pv(*pvs.pop(0))

            # ---- tail ----
            # remaining STs: last blocks of u2/u3, then u4..u7 in full;
            # remaining PVs: u0's last block, then u1..u7 in full. Pace ST
            # emission at 72 STs / 116 PV steps so the scalar engine stays
            # fed without over-buffering exp tiles.
            st_queue = [(u, k) for u in range(4, 8) for k in range(NKT)]
            pv_queue = [(u, k) for u in range(0, 8) for k in range(NKT)]
            n_st, n_pv = len(st_queue) - 32, (6 - 1) * NKT
            st_done = 0
            flushed = False
            for pv_done, pv in enumerate(pv_queue):
                if pv[0] == 7 and pv[1] >= 10:
                    endgame[0] = True
                if pv[0] >= 6 and not flushed:
                    # drain every remaining ST (u6/u7) before their PVs,
                    # weaving drips/posts into the ring-paced ST stream
                    flushed = True
                    k = 0
                    while st_queue:
                        emit_st(*st_queue.pop(0))
                        st_done += 1
                        k += 1
                        if k % 2 == 0:
                            if drip:
                                g = drip.pop(0)
                                emit_d_group(g[0], g[1], d_only=g[2])
                            elif post:
                                pop_post()
                tgt = max((85 * pv_done) // 100, (n_st * pv_done) // n_pv)
                while st_queue and st_done <= tgt:
                    emit_st(*st_queue.pop(0))
                    st_done += 1
                emit_pv(*pv)
                step_ctr[0] += 1
                pop_post()
                if (pv_done % 2 == 1 or pv[0] == 7) and drip:
                    g = drip.pop(0)
                    emit_d_group(g[0], g[1], d_only=g[2])
            i = 0
            while post or drip:
                if post:
                    pop_post(force=True)
                if drip:
                    g = drip.pop(0)
                    emit_d_group(g[0], g[1], evict=("act", "dve")[i % 2],
                                 d_only=g[2])
                    i += 1

    nc.finalize()
    return nc


def get_nc():
    global _NC_CACHE
    if _NC_CACHE is None:
        _NC_CACHE = _build_nc()
    return _NC_CACHE


def make_in_maps(x, W_q, b_q, W_k, b_k, W_v, b_v, W_out, b_out):
    xb = [np.ascontiguousarray(x[b].T).astype(NP_BF16) for b in range(B)]
    in_maps = []
    for c in range(8):
        b, g = divmod(c, 4)
        sl = slice(DG * g, DG * (g + 1))
        in_maps.append({
            "xT": xb[b],
            "wqT": np.ascontiguousarray(W_q[sl, :].T).astype(NP_BF16),
            "wkT": np.ascontiguousarray(W_k[sl, :].T).astype(NP_BF16),
            "wvT": np.ascontiguousarray(W_v[sl, :].T).astype(NP_BF16),
            "woT": np.ascontiguousarray(W_out[:, sl].T).astype(NP_BF16),
            "bq": b_q[sl].reshape(2, 128).astype(np.float32),
            "bk": b_k[sl].reshape(2, 128).astype(np.float32),
        })
    return in_maps


def combine_outputs(outs, W_out, b_out, b_v):
    host_bias = (b_out + b_v @ W_out.T).astype(np.float32)
    y = np.empty((B, S, D), np.float32)
    for b in range(B):
        y[b] = outs[4 * b] + outs[4 * b + 1] + outs[4 * b + 2] + outs[4 * b + 3]
        y[b] += host_bias
    return y


def kernel(x, W_q, b_q, W_k, b_k, W_v, b_v, W_out, b_out):
    x = np.asarray(x, dtype=np.float32)
    args = [np.asarray(a, dtype=np.float32)
            for a in (W_q, b_q, W_k, b_k, W_v, b_v, W_out, b_out)]
    W_q, b_q, W_k, b_k, W_v, b_v, W_out, b_out = args
    nc = get_nc()
    in_maps = make_in_maps(x, W_q, b_q, W_k, b_k, W_v, b_v, W_out, b_out)
    last_err = None
    for attempt in range(3):
        try:
            res = run_bass_kernel_spmd(nc, in_maps, core_ids=list(range(8)))
            break
        except Exception as e:  # transient device-unrecoverable flakes
            last_err = e
            import time
            time.sleep(10)
    else:
        raise last_err
    outs = [np.asarray(r["out"], dtype=np.float32) for r in res.results]
    return combine_outputs(outs, W_out, b_out, b_v)


# revision 48
# speedup vs baseline: 1.3351x; 1.0012x over previous
"""Multi-head self-attention Trainium2 kernel (v2, 166us vs 222us baseline).

Sharding: 8 cores = 2 batches x 4 head-groups. Core c handles batch c//4 and
heads [4g, 4g+4) where g = c%4 (dims [256g, 256g+256) of the 1024 model dim).

Design (timings from the TimelineSim cost model, which charges matmuls by
moving-operand columns only):
  - All matmul operands bf16: inputs cast on host (halves input DMA), Q/K/V
    built via fp32-PSUM projections and rounded once. Output DMA'd as bf16.
  - Score matmuls contract over each head's 64 partitions directly
    (K/Q packed two heads per 128 partitions; no zero-padded K copy).
  - PV flipped: exp(S^T) is the stationary operand, the ones-augmented bf16
    V tile the moving one, so each accumulation step streams 65 columns
    instead of 512 (PV tensor time 54.6us -> 27.7us). ctx accumulates in
    natural [token, dim] layout in one [128, 8, 128] PSUM tile per unit
    (eight 65-wide groups at 512B offsets, exactly one start/stop per 2KB
    PSUM bank; column 64 collects the softmax denominator). A PE transpose
    against a bf16 identity restores ctx^T for the out-projection.
  - The scalar engine's exp stream (~133us busy) is the critical chain:
    score/exp steps for the first four q-chunk units interleave into the
    projection phase as their K/Q chunks land (64 of 128 exp tiles retired
    before projections finish), the tail paces one ST per ~1.5 PV steps,
    and a 60-deep exp-tile pool prevents slot-reuse stalls.
  - PE warmup matmuls bridge the initial DMA wait (the cost model's p-state
    ramp punishes any idle gap before full clock is reached).
  - Normalization reciprocals/multiplies, ctx transposes+copies, and
    out-projection drips are spread across PV steps (transposes age-gated
    two steps behind their stage multiplies so the PE never waits on DVE);
    the endgame rotates evictions over DVE and the scalar engine (gpsimd
    cannot touch PSUM).

Host: shards/transposes inputs, sums the 4 partial outputs per batch and adds
b_out + b_v @ W_out^T (the V-bias contribution commutes through softmax since
attention rows sum to 1).
"""

import numpy as np

import concourse.bacc as bacc
import concourse.mybir as mybir
from concourse.masks import make_identity
from concourse.tile import TileContext
from concourse.bass_utils import run_bass_kernel_spmd

AF = mybir.ActivationFunctionType
I16 = mybir.dt.int16
# Schraudolph exp in bf16-bit space: bf16bits(exp(s)) ~= int16(A*s + B)
SCH_A = 128.0 / float(np.log(2.0))
SCH_B = 127.0 * 128.0 - 7.0 + 0.5
SCH_SET = None  # set below after os import
ALU = mybir.AluOpType
F32 = mybir.dt.float32
BF16 = mybir.dt.bfloat16
NP_BF16 = mybir.dt.np(BF16)

B, S, D, H, DH = 2, 2048, 1024, 16, 64
DG = 256          # dims per head-group (4 heads)
TC = 512          # token / query chunk
NTC = S // TC     # 4
NKT = S // 128    # 16 key tiles

_NC_CACHE = None


def _build_nc():
    nc = bacc.Bacc("TRN2", target_bir_lowering=False, debug=False)

    xT = nc.dram_tensor("xT", [D, S], BF16, kind="ExternalInput")
    wq = nc.dram_tensor("wqT", [D, DG], BF16, kind="ExternalInput")
    wk = nc.dram_tensor("wkT", [D, DG], BF16, kind="ExternalInput")
    wv = nc.dram_tensor("wvT", [D, DG], BF16, kind="ExternalInput")
    wo = nc.dram_tensor("woT", [DG, D], BF16, kind="ExternalInput")
    bq = nc.dram_tensor("bq", [2, 128], F32, kind="ExternalInput")
    bk = nc.dram_tensor("bk", [2, 128], F32, kind="ExternalInput")
    out = nc.dram_tensor("out", [S, D], BF16, kind="ExternalOutput")

    with TileContext(nc) as tc:
        with (
            tc.tile_pool(name="const", bufs=1) as constp,
            tc.tile_pool(name="xt", bufs=2) as xtp,
            tc.tile_pool(name="expst", bufs=60) as expp,
            tc.tile_pool(name="stg", bufs=12) as stgp,
            tc.tile_pool(name="small", bufs=4) as smallp,
            tc.tile_pool(name="outp", bufs=3) as outp,
            tc.tile_pool(name="s_ps", bufs=2, space="PSUM") as sps,
            tc.tile_pool(name="p_ps", bufs=2, space="PSUM") as pps,
            tc.tile_pool(name="c_ps", bufs=1, space="PSUM") as ctxps,
        ):
            # ---- persistent tiles ----
            wq_s = constp.tile([128, 8, DG], BF16)
            wk_s = constp.tile([128, 8, DG], BF16)
            wv_s = constp.tile([128, 8, DG], BF16)
            bq_s = constp.tile([128, 2], F32)
            bk_s = constp.tile([128, 2], F32)
            xTr = xT[:, :].rearrange("(k p) t -> p k t", p=128)
            xt0 = xtp.tile([128, 8, TC], BF16, name="xt0", tag="xt")
            wqr = wq[:, :].rearrange("(k p) m -> p k m", p=128)
            nc.sync.dma_start(out=wq_s[:, 0:4, :], in_=wqr[:, 0:4, :])
            nc.sync.dma_start(out=xt0[:, 0:4, :], in_=xTr[:, 0:4, 0:TC])
            nc.sync.dma_start(out=wq_s[:, 4:8, :], in_=wqr[:, 4:8, :])
            nc.sync.dma_start(out=wk_s, in_=wk[:, :].rearrange("(k p) m -> p k m", p=128))
            nc.sync.dma_start(out=xt0[:, 4:8, :], in_=xTr[:, 4:8, 0:TC])
            nc.sync.dma_start(out=bq_s, in_=bq[:, :].rearrange("t p -> p t"))
            nc.sync.dma_start(out=bk_s, in_=bk[:, :].rearrange("t p -> p t"))
            nc.sync.dma_start(out=wv_s, in_=wv[:, :].rearrange("(k p) m -> p k m", p=128))
            wo_s = constp.tile([128, 2, D], BF16)
            nc.sync.dma_start(out=wo_s, in_=wo[:, :].rearrange("(k p) m -> p k m", p=128))

            # QT/KT: heads of dtile d packed along partitions (head hh in
            # partitions [64hh, 64hh+64)); bf16
            QT_s = constp.tile([128, 2, S], BF16)
            KT_s = constp.tile([128, 2, S], BF16)
            ctxT_s = constp.tile([128, 2, S], BF16)
            # V natural layout, bf16, ones-column at index 64 per head
            Vg_s = constp.tile([128, NKT, 4, 65], BF16)
            nc.vector.memset(Vg_s[:, :, :, 64:65], 1.0)
            ident = constp.tile([128, 128], BF16)
            make_identity(nc, ident)

            # warm the PE clock while the initial DMAs stream in
            warm = constp.tile([128, TC], BF16)
            nc.vector.memset(warm, 1.0)
            wps = pps.tile([128, TC], F32, tag="pt", name="wps")
            for _ in range(8):
                nc.tensor.matmul(wps, lhsT=warm[:, 0:128], rhs=warm,
                                 start=True, stop=True)

            # ---- projection group emitters (q/k transposed, v natural) ----
            def emit_q_group(d, tci, xt):
                tsl = slice(tci * TC, (tci + 1) * TC)
                dsl = slice(d * 128, (d + 1) * 128)
                psq = pps.tile([128, TC], F32, tag="pt", name="psq")
                for k in range(8):
                    nc.tensor.matmul(psq, lhsT=wq_s[:, k, dsl], rhs=xt[:, k, :],
                                     start=(k == 0), stop=(k == 7))
                nc.vector.tensor_scalar(QT_s[:, d, tsl], psq,
                                        scalar1=bq_s[:, d:d + 1], scalar2=0.125,
                                        op0=ALU.add, op1=ALU.mult)

            def emit_k_group(d, tci, xt):
                tsl = slice(tci * TC, (tci + 1) * TC)
                dsl = slice(d * 128, (d + 1) * 128)
                psk = pps.tile([128, TC], F32, tag="pt", name="psk")
                for k in range(8):
                    nc.tensor.matmul(psk, lhsT=wk_s[:, k, dsl], rhs=xt[:, k, :],
                                     start=(k == 0), stop=(k == 7))
                nc.vector.tensor_scalar(KT_s[:, d, tsl], psk,
                                        scalar1=bk_s[:, d:d + 1],
                                        scalar2=None, op0=ALU.add)

            def emit_v_group(tci, tt, xt):
                ti = tci * 4 + tt
                psv = pps.tile([128, TC], F32, tag="pt", name="psv")
                for k in range(8):
                    nc.tensor.matmul(psv[:, 0:DG],
                                     lhsT=xt[:, k, tt * 128:(tt + 1) * 128],
                                     rhs=wv_s[:, k, :], start=(k == 0), stop=(k == 7))
                # single strided copy: psv natural [t, 4*64] -> Vg[:, ti, h, 0:64]
                nc.vector.tensor_copy(Vg_s[:, ti, 0:4, 0:64],
                                      psv[:, 0:DG].rearrange("p (h e) -> p h e", h=4))

            # ---- attention emitters ----
            units = [(qc, d) for qc in range(NTC) for d in range(2)]
            ex_of = {}     # (u, kt) -> exp tile (bf16 SBUF)
            cps_of = {}    # u -> PSUM accum tile [128, 8, 128]
            drip = []      # pending out-projection groups

            def emit_st(u, kt):
                qc, d = units[u]
                qsl = slice(qc * TC, (qc + 1) * TC)
                ksl = slice(kt * 128, (kt + 1) * 128)
                sp = sps.tile([128, 2, TC], F32, tag="s", name="sp")
                for hh in range(2):
                    p0 = 64 * hh
                    nc.tensor.matmul(sp[:, hh, :],
                                     lhsT=KT_s[p0:p0 + 64, d, ksl],
                                     rhs=QT_s[p0:p0 + 64, d, qsl],
                                     start=True, stop=True)
                if (u, kt) in SCH_SET:
                    exi = expp.tile([128, 2, TC], I16, tag="ex", name="exi")
                    nc.vector.tensor_scalar(exi, sp, scalar1=SCH_A,
                                            scalar2=SCH_B,
                                            op0=ALU.mult, op1=ALU.add)
                    ex_of[(u, kt)] = exi.bitcast(BF16)
                else:
                    ex = expp.tile([128, 2, TC], BF16, tag="ex", name="ex")
                    nc.scalar.activation(ex, sp, AF.Exp)
                    ex_of[(u, kt)] = ex

            def emit_pv(u, kt):
                qc, d = units[u]
                ex = ex_of.pop((u, kt))
                if kt == 0:
                    cps_of[u] = ctxps.tile([128, 8, 128], F32, tag="c", name="cps")
                cps = cps_of[u]
                # cps is two PSUM banks (groups 0-3 / 4-7). start=True marks a
                # whole 2KB bank pending-zero, so exactly one start per bank
                # (first group) and one stop per bank (last group).
                for qs in range(4):
                    for hh in range(2):
                        h = 2 * d + hh
                        g = 2 * qs + hh
                        nc.tensor.matmul(cps[:, g, 0:65],
                                         lhsT=ex[:, hh, qs * 128:(qs + 1) * 128],
                                         rhs=Vg_s[:, kt, h, :],
                                         start=(kt == 0 and g % 4 == 0),
                                         stop=(kt == NKT - 1 and g % 4 == 3))
                if kt == NKT - 1:
                    emit_norm(u)

            post = []      # deferred (unit, transpose+copy closure) steps

            def emit_norm(u):
                qc, d = units[u]
                cps = cps_of.pop(u)
                rec = smallp.tile([128, 8, 1], F32, tag="rec", name="rec")
                nc.vector.reciprocal(rec, cps[:, :, 64:65])
                stages = []
                for qs in range(4):
                    stage = stgp.tile([128, 2, 64], BF16, tag="stg", name="stage")
                    for hh in range(2):
                        nc.vector.tensor_scalar(stage[:, hh, :],
                                                cps[:, 2 * qs + hh, 0:64],
                                                scalar1=rec[:, 2 * qs + hh, :],
                                                scalar2=None, op0=ALU.mult)
                    stages.append(stage)

                def mk(qs, stage):
                    def go():
                        tps = pps.tile([128, 128], BF16, tag="pt", name="tps")
                        nc.tensor.transpose(
                            tps, stage[:, :, :].rearrange("p a b -> p (a b)"),
                            ident)
                        qsl = slice(qc * TC + qs * 128, qc * TC + (qs + 1) * 128)
                        if endgame[0] and qs % 2 == 0:
                            nc.scalar.copy(ctxT_s[:, d, qsl], tps)
                        else:
                            nc.vector.tensor_copy(ctxT_s[:, d, qsl], tps)
                    return go
                for qs in range(4):
                    post.append((step_ctr[0], u, qs, mk(qs, stages[qs])))

            done_posts = {}
            released = set()

            def _maybe_release_drips(u, qs=None):
                qc, d = units[u]
                other = u - 1 if d == 1 else u + 1
                for q in ([qs] if qs is not None else range(4)):
                    if (q in done_posts.get(u, set())
                            and q in done_posts.get(other, set())
                            and (u, q) not in released):
                        released.add((u, q))
                        released.add((other, q))
                        drip.extend(((qc * 4 + q, oc, None) for oc in range(2)))

            step_ctr = [0]

            def pop_post(force=False):
                if not post:
                    return
                if not force and step_ctr[0] - post[0][0] < 2:
                    return
                _, u, qs, go = post.pop(0)
                go()
                done_posts.setdefault(u, set()).add(qs)
                _maybe_release_drips(u, qs)

            endgame = [False]
            dripn = [0]

            def emit_d_group(tt, oc, evict="dve", d_only=None):
                psl = slice(tt * 128, (tt + 1) * 128)
                osl = slice(oc * TC, (oc + 1) * TC)
                dripn[0] += 1
                if endgame[0]:
                    evict = ("dve", "act")[dripn[0] % 2]
                if endgame[0] and dripn[0] % 2 == 0:
                    po = ctxps.tile([128, 8, 128], F32, tag="c",
                                    name="poc")[:, 0:4, :].rearrange("p a b -> p (a b)")
                else:
                    po = pps.tile([128, TC], F32, tag="pt", name="po")
                for i, d in enumerate((0, 1)):
                    nc.tensor.matmul(po, lhsT=ctxT_s[:, d, psl],
                                     rhs=wo_s[:, d, osl],
                                     start=(i == 0), stop=(i == 1))
                ot = outp.tile([128, TC], BF16)
                if evict == "act":
                    nc.scalar.copy(ot, po)
                else:
                    nc.vector.tensor_copy(ot, po)
                nc.sync.dma_start(out=out[psl, osl], in_=ot)

            # ---- fused projection + early-attention phase ----
            # Per x-chunk tci: project Q/K/V with ST/exp steps interleaved
            # between groups. STs are limited to what the scalar engine can
            # absorb and to Q/K chunks that exist. Only u0's PV trails here
            # (single PSUM accumulator live).
            early_st = {
                0: [(0, k) for k in range(4)] + [(1, k) for k in range(4)],
                1: [(0, k) for k in range(4, 8)] + [(1, k) for k in range(4, 8)]
                   + [(2, k) for k in range(4)] + [(3, k) for k in range(4)],
                2: [(0, k) for k in range(8, 12)] + [(1, k) for k in range(8, 12)]
                   + [(2, k) for k in range(4, 8)] + [(3, k) for k in range(4, 8)],
                3: [(0, k) for k in range(12, 16)] + [(1, k) for k in range(12, 16)]
                   + [(2, k) for k in range(8, 16)] + [(3, k) for k in range(8, 16)],
            }
            early_pv = {}
            for tci in range(NTC):
                if tci == 0:
                    xt = xt0
                else:
                    xt = xtp.tile([128, 8, TC], BF16, tag="xt", name="xt")
                    nc.sync.dma_start(out=xt, in_=xTr[:, :, tci * TC:(tci + 1) * TC])
                # old-kt STs (kt from previous chunks) may run right away;
                # new-kt STs need this chunk's K groups emitted first.
                sts = list(early_st[tci])
                old_sts = [s for s in sts if s[1] < 4 * tci]
                # new-kt STs split by which K d-group they need
                new0 = [s for s in sts if s[1] >= 4 * tci and units[s[0]][1] == 0]
                new1 = [s for s in sts if s[1] >= 4 * tci and units[s[0]][1] == 1]
                pvs = list(early_pv.get(tci, []))
                ready = [False, False]

                def weave(n_st):
                    for _ in range(n_st):
                        if old_sts:
                            emit_st(*old_sts.pop(0))
                        elif ready[0] and new0:
                            emit_st(*new0.pop(0))
                        elif ready[1] and new1:
                            emit_st(*new1.pop(0))
                    if pvs:
                        emit_pv(*pvs.pop(0))
                        step_ctr[0] += 1
                    elif post:
                        pop_post()

                nflush = 0
                for s in list(old_sts):
                    if nflush >= 3:
                        break
                    if units[s[0]][0] < tci:
                        old_sts.remove(s)
                        emit_st(*s)
                        nflush += 1
                emit_q_group(0, tci, xt)
                emit_k_group(0, tci, xt)
                ready[0] = True
                weave(2)
                emit_q_group(1, tci, xt)
                weave(2)
                emit_k_group(1, tci, xt)
                ready[1] = True
                weave(2)
                for tt in range(4):
                    emit_v_group(tci, tt, xt)
                    weave(2)
                while old_sts or new0 or new1:
                    weave(2)
                while pvs:
                    emit_pv(*pvs.pop(0))

            # ---- tail ----
            # remaining STs: last blocks of u2/u3, then u4..u7 in full;
            # remaining PVs: u0's last block, then u1..u7 in full. Pace ST
            # emission at 72 STs / 116 PV steps so the scalar engine stays
            # fed without over-buffering exp tiles.
            st_queue = [(u, k) for u in range(4, 8) for k in range(NKT)]
            pv_queue = [(u, k) for u in range(0, 8) for k in range(NKT)]
            n_st, n_pv = len(st_queue) - 32, (6 - 1) * NKT
            st_done = 0
            flushed = False
            for pv_done, pv in enumerate(pv_queue):
                if pv[0] == 7 and pv[1] >= 10:
                    endgame[0] = True
                if pv[0] >= 6 and not flushed:
                    # drain every remaining ST (u6/u7) before their PVs,
                    # weaving drips/posts into the ring-paced ST stream
                    flushed = True
                    k = 0
                    while st_queue:
                        emit_st(*st_queue.pop(0))
                        st_done += 1
                        k += 1
                        if k % 2 == 0:
                            if drip:
                                g = drip.pop(0)
                                emit_d_group(g[0], g[1], d_only=g[2])
                            elif post:
                                pop_post()
                tgt = max((85 * pv_done) // 100, (n_st * pv_done) // n_pv)
                while st_queue and st_done <= tgt:
                    emit_st(*st_queue.pop(0))
                    st_done += 1
                emit_pv(*pv)
                step_ctr[0] += 1
                pop_post()
                if (pv_done % 2 == 1 or pv[0] == 7) and drip:
                    g = drip.pop(0)
                    emit_d_group(g[0], g[1], d_only=g[2])
            i = 0
            while post or drip:
                if post:
                    pop_post(force=True)
                if drip:
                    g = drip.pop(0)
                    emit_d_group(g[0], g[1], evict=("act", "dve")[i % 2],
                                 d_only=g[2])
                    i += 1

    nc.finalize()
    return nc


def get_nc():
    global _NC_CACHE
    if _NC_CACHE is None:
        _NC_CACHE = _build_nc()
    return _NC_CACHE


def make_in_maps(x, W_q, b_q, W_k, b_k, W_v, b_v, W_out, b_out):
    xb = [np.ascontiguousarray(x[b].T).astype(NP_BF16) for b in range(B)]
    in_maps = []
    for c in range(8):
        b, g = divmod(c, 4)
        sl = slice(DG * g, DG * (g + 1))
        in_maps.append({
            "xT": xb[b],
            "wqT": np.ascontiguousarray(W_q[sl, :].T).astype(NP_BF16),
            "wkT": np.ascontiguousarray(W_k[sl, :].T).astype(NP_BF16),
            "wvT": np.ascontiguousarray(W_v[sl, :].T).astype(NP_BF16),
            "woT": np.ascontiguousarray(W_out[:, sl].T).astype(NP_BF16),
            "bq": b_q[sl].reshape(2, 128).astype(np.float32),
            "bk": b_k[sl].reshape(2, 128).astype(np.float32),
        })
    return in_maps


def combine_outputs(outs, W_out, b_out, b_v):
    host_bias = (b_out + b_v @ W_out.T).astype(np.float32)
    y = np.empty((B, S, D), np.float32)
    for b in range(B):
        y[b] = outs[4 * b] + outs[4 * b + 1] + outs[4 * b + 2] + outs[4 * b + 3]
        y[b] += host_bias
    return y


def kernel(x, W_q, b_q, W_k, b_k, W_v, b_v, W_out, b_out):
    x = np.asarray(x, dtype=np.float32)
    args = [np.asarray(a, dtype=np.float32)
            for a in (W_q, b_q, W_k, b_k, W_v, b_v, W_out, b_out)]
    W_q, b_q, W_k, b_k, W_v, b_v, W_out, b_out = args
    nc = get_nc()
    in_maps = make_in_maps(x, W_q, b_q, W_k, b_k, W_v, b_v, W_out, b_out)
    last_err = None
    for attempt in range(3):
        try:
            res = run_bass_kernel_spmd(nc, in_maps, core_ids=list(range(8)))
            break
        except Exception as e:  # transient device-unrecoverable flakes
            last_err = e
            import time
            time.sleep(10)
    else:
        raise last_err
    outs = [np.asarray(r["out"], dtype=np.float32) for r in res.results]
    return combine_outputs(outs, W_out, b_out, b_v)
